# revision 25
# baseline (speedup 1.0000x reference)
"""Trainium2 Bass kernel for nn_Attention_82892868813208.

Full attention layer: QKV proj + RoPE + causal softmax attention + output proj.
  B=2, S=2048, HIDDEN=2048, HEADS=32, HD=64, causal.

Sharding (8 cores): core c = (batch b=c//4, head-group g=c%4 of 8 heads).
Each core computes QKV+RoPE+attention for its 8 heads on its batch, then a
partial output projection (w_o row-shard); a device-side ReduceScatter over
the 4 cores of each batch sums the partials and scatters by output-feature
rows (chunked by token quarters so it pipelines behind the projection).
Host concatenates + transposes.

Layouts (per core, on device):
  hidT   (2048 hid, 2048 tok) f16  -- hidden transposed on host
  w_qkv  (2048, 1536) f16 cols = [Q 8 heads | K 8 heads | V 8 heads]
  Q',K'  kept transposed: (64 d, 2048 tok) per head, 2 heads per 128-partition
  V      natural (tok, 64) per head + a ones column (softmax denominator)
  scores computed transposed: S^T (keys on partitions, queries free), so the
         softmax normalizer comes out of the AV matmul's ones column and all
         reductions stay on the free axis.

Schedule: head-pair p's QKV (+V on p==0) streams hidden per 512-token chunk,
then its two heads' attention runs; the next pair's QKV matmuls overlap the
exp/softmax of the current pair.  Attention per head iterates query-chunk
outer / key-block-pair inner so only ~2 PSUM banks of AV accumulators are
live at a time.
"""

import sys

sys.path.insert(0, "/opt/trn_rl_repo")

import numpy as np

import concourse.bass as bass
import concourse.mybir as mybir
from concourse import bacc
import concourse.tile as tile
from concourse.bass_utils import run_bass_kernel_spmd

P = 128
S = 2048
HID = 2048
HD = 64
HPG = 8          # heads per group (per core)
KB = HID // P    # 16 contraction blocks
NT = 4           # 512-token chunks
TC = 512
QKV_LOCAL = 3 * HPG * HD  # 1536
F16 = mybir.dt.float16
F32 = mybir.dt.float32

# module-level knobs for test.py
TRACE = False
TRACE_KW = {}
_LAST_RESULTS = None


def build_program(with_rs=True):
    # with_rs: True = real (ReduceScatter); False = full partial out (sim
    # correctness); "model" = RS-shaped traffic without collectives, for
    # single-core TimelineSim cost analysis.
    model = with_rs == "model"
    nc = bacc.Bacc(trn_type="TRN2", num_devices=8)

    hidT = nc.dram_tensor("hidT", [HID, S], F16, kind="ExternalInput")
    wqkv = nc.dram_tensor("wqkv", [HID, QKV_LOCAL], F16, kind="ExternalInput")
    wo = nc.dram_tensor("wo", [HPG * HD, HID], F16, kind="ExternalInput")
    cosb = nc.dram_tensor("cosb", [P, S], F16, kind="ExternalInput")
    sinb = nc.dram_tensor("sinb", [P, S], F16, kind="ExternalInput")
    r2t = nc.dram_tensor("r2t", [P, P], F16, kind="ExternalInput")
    maskbig = nc.dram_tensor("maskbig", [P, 640], F16, kind="ExternalInput")
    y = nc.dram_tensor(
        "y", [NT, HID if with_rs is False else HID // 4, TC], F32,
        kind="ExternalOutput"
    )

    with tile.TileContext(nc) as tc:
        with (
            tc.tile_pool(name="const", bufs=1) as cpool,
            tc.tile_pool(name="hid", bufs=2) as hidp,
            tc.tile_pool(name="tmps", bufs=2) as tmps,
            tc.tile_pool(name="pt", bufs=2) as ptp,
            tc.tile_pool(name="fino", bufs=6) as finop,
            # PSUM: 8 banks static: a=2x1 (qkv/V/rot/fin), av=2x1, b=2x2 (sc)
            tc.tile_pool(name="psa", bufs=2, space="PSUM") as psa,
            tc.tile_pool(name="psav", bufs=2, space="PSUM") as psav,
            tc.tile_pool(name="psb", bufs=2, space="PSUM") as psb,
            tc.tile_pool(name="dram", bufs=1, space="DRAM") as dramp,
        ):
            # ---- persistent tiles ----
            # first hidden chunk and the qkv weights feed the first matmul
            # chain; interleave their DMAs so neither serializes the other
            # on the sync queue.
            hidT_r0 = hidT.ap().rearrange("(ko ki) t -> ki ko t", ki=P)
            hid_t0 = hidp.tile([P, KB, TC], F16, tag="hid", name="hid_t0")
            w_sb = cpool.tile([P, KB, QKV_LOCAL], F16, name="w_sb")
            wqkv_r = wqkv.ap().rearrange("(ko ki) f -> ki ko f", ki=P)
            for kb in range(KB):
                nc.sync.dma_start(
                    hid_t0[:, kb, :], hidT_r0[:, kb, 0:TC]
                )
                nc.sync.dma_start(w_sb[:, kb, :], wqkv_r[:, kb, :])
            cos_sb = cpool.tile([P, S], F16, name="cos_sb")
            nc.sync.dma_start(cos_sb[:], cosb.ap())
            sin_sb = cpool.tile([P, S], F16, name="sin_sb")
            nc.sync.dma_start(sin_sb[:], sinb.ap())
            r2_sb = cpool.tile([P, P], F16, name="r2_sb")
            nc.sync.dma_start(r2_sb[:], r2t.ap())
            mask_sb = cpool.tile([P, 640], F16, name="mask_sb")
            nc.sync.dma_start(mask_sb[:], maskbig.ap())
            ones_sb = cpool.tile([P, HD], F16, name="ones_sb")
            nc.gpsimd.memset(ones_sb[:], 1.0)

            qk_sb = cpool.tile([P, 8, S], F16, name="qk_sb")
            v_sb = cpool.tile([P, KB, 65 * HPG], F16, name="v_sb")
            nc.gpsimd.memset(v_sb[:], 1.0)
            outcat_sb = cpool.tile([P, 4, S], F16, name="outcat_sb")
            recz_sb = cpool.tile([P, S], F16, name="recz_sb")
            wo_sb = cpool.tile([P, 4, HID], F16, name="wo_sb")

            pdt = F32 if with_rs is False else F16
            partial = [
                dramp.tile([HID, TC], pdt, name=f"partial{i}")
                for i in range(NT)
            ]
            if with_rs is True:
                rs_out = [
                    dramp.tile([HID // 4, TC], F16, name=f"rs_out{i}")
                    for i in range(NT)
                ]
            elif model:
                rs_out = [partial[i][0:HID // 4, :] for i in range(NT)]

            hidT_r = hidT.ap().rearrange("(ko ki) t -> ki ko t", ki=P)

            def qkv_block(m, t, hid_t):
                """QKV m-block (2 heads' Q or K, transposed) for token chunk t,
                with RoPE, into qk_sb[:, m, 512t:...]."""
                ts = slice(t * TC, (t + 1) * TC)
                ps = psa.tile([P, TC], F32, tag="a", name="psqk")
                for kb in range(KB):
                    nc.tensor.matmul(
                        ps[:],
                        lhsT=w_sb[:, kb, m * P:(m + 1) * P],
                        rhs=hid_t[:, kb, :],
                        start=(kb == 0),
                        stop=(kb == KB - 1),
                    )
                qtmp = tmps.tile([P, TC], F16, tag="qtmp")
                nc.scalar.copy(qtmp[:], ps[:])
                rot = psa.tile([P, TC], F32, tag="a", name="rot")
                nc.tensor.matmul(rot[:], lhsT=r2_sb[:], rhs=qtmp[:])
                t1 = tmps.tile([P, TC], F16, tag="t1")
                nc.vector.tensor_tensor(
                    t1[:], ps[:], cos_sb[:, ts], mybir.AluOpType.mult
                )
                t2 = tmps.tile([P, TC], F16, tag="t2")
                nc.vector.tensor_tensor(
                    t2[:], rot[:], sin_sb[:, ts], mybir.AluOpType.mult
                )
                nc.vector.tensor_tensor(
                    qk_sb[:, m, ts], t1[:], t2[:], mybir.AluOpType.add
                )

            def v_block(t, hid_t):
                """V (all 8 heads, natural token-major) for token chunk t."""
                for tb in range(4):
                    tbi = 4 * t + tb
                    pv = psa.tile([P, TC], F32, tag="a", name="psv")
                    for kb in range(KB):
                        nc.tensor.matmul(
                            pv[:],
                            lhsT=hid_t[:, kb, tb * P:(tb + 1) * P],
                            rhs=w_sb[:, kb, 2 * HPG * HD:3 * HPG * HD],
                            start=(kb == 0),
                            stop=(kb == KB - 1),
                        )
                    v_dst = v_sb[:, tbi, :].rearrange("p (h c) -> p h c", c=65)
                    nc.scalar.copy(
                        v_dst[:, :, 0:HD],
                        pv[:].rearrange("p (h c) -> p h c", c=HD),
                    )

            def attention_head(h):
                ph = 64 * (h % 2)
                qb = h // 2
                kblk = 4 + h // 2
                for c in range(4):
                    av = psav.tile([65, TC], F32, tag="av", name="av")
                    jtop = 4 * c + 3  # last key block for this query chunk
                    for J0 in range(0, jtop + 1, 2):
                        pair = [J for J in (J0, J0 + 1) if J <= jtop]
                        sc = psb.tile([P, 1024], F32, tag="b", name="sc")
                        pt = ptp.tile([P, 1024], F16, tag="pt")
                        segs = []  # valid (exp) segments within the 1024 tile
                        for i, J in enumerate(pair):
                            # pad: queries < 128J are fully masked
                            off = P * (J % 4) if J // 4 == c else 0
                            lo = TC * i + off
                            hi = TC * (i + 1)
                            nc.tensor.matmul(
                                sc[:, lo:hi],
                                lhsT=qk_sb[ph:ph + 64, kblk,
                                           J * P:(J + 1) * P],
                                rhs=qk_sb[ph:ph + 64, qb,
                                          TC * c + off:TC * (c + 1)],
                                start=True,
                                stop=True,
                            )
                            if J // 4 == c:  # diagonal block: causal mask
                                nc.vector.tensor_tensor(
                                    sc[:, lo:lo + P],
                                    sc[:, lo:lo + P],
                                    mask_sb[:, 384:384 + P],
                                    mybir.AluOpType.add,
                                )
                            if off:
                                nc.gpsimd.memset(pt[:, TC * i:lo], 0.0)
                            if segs and segs[-1][1] == lo:
                                segs[-1] = (segs[-1][0], hi)
                            else:
                                segs.append((lo, hi))
                        for (lo, hi) in segs:
                            nc.scalar.activation(
                                pt[:, lo:hi], sc[:, lo:hi],
                                mybir.ActivationFunctionType.Exp,
                                scale=0.125,
                            )
                        for i, J in enumerate(pair):
                            nc.tensor.matmul(
                                av[:],
                                lhsT=v_sb[:, J, 65 * h:65 * h + 65],
                                rhs=pt[:, TC * i:TC * (i + 1)],
                                start=(J == 0),
                                stop=(J == jtop),
                            )
                    # normalize: 1/Z (ones-column row), PE-broadcast, multiply
                    cs = slice(c * TC, (c + 1) * TC)
                    with nc.allow_low_precision(
                        reason="1/Z fed to f16 broadcast matmul; f16 suffices"
                    ):
                        nc.vector.reciprocal(recz_sb[64:65, cs], av[64:65, :])
                    bc = psb.tile([P, 1024], F32, tag="b", name="bc")
                    nc.tensor.matmul(
                        bc[0:64, 0:TC],
                        lhsT=ones_sb[64:65, 0:HD],
                        rhs=recz_sb[64:65, cs],
                    )
                    bcs = tmps.tile([64, TC], F16, tag="bcs")
                    nc.scalar.copy(bcs[:], bc[0:64, 0:TC])
                    nc.vector.tensor_tensor(
                        outcat_sb[ph:ph + 64, qb, cs],
                        av[0:64, :],
                        bcs[:],
                        mybir.AluOpType.mult,
                    )

            # ---- interleaved QKV + attention, one head pair at a time ----
            for p in range(4):
                for t in range(NT):
                    if p == 0 and t == 0:
                        hid_t = hid_t0
                    else:
                        hid_t = hidp.tile([P, KB, TC], F16, tag="hid")
                        for kg in range(4):
                            nc.sync.dma_start(
                                hid_t[:, 4 * kg:4 * (kg + 1), :],
                                hidT_r[:, 4 * kg:4 * (kg + 1),
                                       t * TC:(t + 1) * TC],
                            )
                    qkv_block(p, t, hid_t)      # Q pair p
                    qkv_block(4 + p, t, hid_t)  # K pair p
                    if p == 0:
                        v_block(t, hid_t)
                if p == 0:
                    wo_r = wo.ap().rearrange("(co ci) e -> ci co e", ci=P)
                    nc.sync.dma_start(wo_sb[:], wo_r)
                attention_head(2 * p)
                attention_head(2 * p + 1)

            # ---- partial output projection, chunked ReduceScatter ----
            for ca in range(NT):
                for m in range(KB):
                    fin = psa.tile([P, TC], F32, tag="a", name="fin")
                    for kb in range(4):
                        nc.tensor.matmul(
                            fin[:],
                            lhsT=wo_sb[:, kb, m * P:(m + 1) * P],
                            rhs=outcat_sb[:, kb, ca * TC:(ca + 1) * TC],
                            start=(kb == 0),
                            stop=(kb == 3),
                        )
                    fo = finop.tile([P, TC], pdt, tag="fino")
                    nc.vector.tensor_copy(out=fo[:], in_=fin[:])
                    nc.scalar.dma_start(
                        partial[ca][m * P:(m + 1) * P, :], fo[:]
                    )
                if with_rs is True:
                    nc.gpsimd.collective_compute(
                        "ReduceScatter",
                        mybir.AluOpType.add,
                        replica_groups=[[0, 1, 2, 3], [4, 5, 6, 7]],
                        ins=[partial[ca][:]],
                        outs=[rs_out[ca][:]],
                    )
                    rs_ap = rs_out[ca][:]
                    # convert f16 ReduceScatter result to the f32 output
                    rsb = finop.tile([P, 4, TC], F16, tag="rsb", bufs=1)
                    nc.sync.dma_start(
                        rsb[:],
                        rs_ap.rearrange("(ro ri) t -> ri ro t", ri=P),
                    )
                    ycv = finop.tile([P, 4, TC], F32, tag="ycv", bufs=1)
                    nc.vector.tensor_copy(out=ycv[:], in_=rsb[:])
                    nc.sync.dma_start(
                        y.ap()[ca].rearrange("(ro ri) t -> ri ro t", ri=P),
                        ycv[:],
                    )
                elif model:
                    rs_ap = rs_out[ca]
                    rsb = finop.tile([P, 4, TC], F16, tag="rsb", bufs=1)
                    nc.sync.dma_start(
                        rsb[:],
                        rs_ap.rearrange("(ro ri) t -> ri ro t", ri=P),
                    )
                    ycv = finop.tile([P, 4, TC], F32, tag="ycv", bufs=1)
                    nc.vector.tensor_copy(out=ycv[:], in_=rsb[:])
                    nc.sync.dma_start(
                        y.ap()[ca].rearrange("(ro ri) t -> ri ro t", ri=P),
                        ycv[:],
                    )
                else:
                    nc.sync.dma_start(y.ap()[ca], partial[ca][:])

    nc.compile()
    return nc


def make_in_maps(hidden_states, cos, sin, w_qkv, w_o):
    hs = np.asarray(hidden_states, dtype=np.float32)
    cos = np.asarray(cos, dtype=np.float32)
    sin = np.asarray(sin, dtype=np.float32)
    wq = np.asarray(w_qkv, dtype=np.float32)
    wo = np.asarray(w_o, dtype=np.float32)

    cosT = cos.T  # (64, S)
    sinT = sin.T
    cosB = np.concatenate([cosT, cosT], axis=0).astype(np.float16)
    sinB = np.concatenate([sinT, sinT], axis=0).astype(np.float16)

    R = np.zeros((HD, HD), dtype=np.float32)
    R[:32, 32:] = -np.eye(32, dtype=np.float32)
    R[32:, :32] = np.eye(32, dtype=np.float32)
    R2T = np.zeros((P, P), dtype=np.float32)
    R2T[:HD, :HD] = R.T
    R2T[HD:, HD:] = R.T
    R2T = R2T.astype(np.float16)

    jj = np.arange(P)[:, None]
    cc = np.arange(640)[None, :]
    maskbig = np.where(jj <= cc - 384, 0.0, -30000.0).astype(np.float16)

    in_maps = []
    for c in range(8):
        b, g = divmod(c, 4)
        h0 = HPG * g
        hidT = np.ascontiguousarray(hs[b].T).astype(np.float16)
        qc = wq[:, HD * h0:HD * (h0 + HPG)]
        kc = wq[:, HD * (32 + h0):HD * (32 + h0 + HPG)]
        vc = wq[:, HD * (64 + h0):HD * (64 + h0 + HPG)]
        w_local = np.concatenate([qc, kc, vc], axis=1).astype(np.float16)
        wo_local = np.ascontiguousarray(
            wo[HD * h0:HD * (h0 + HPG), :]
        ).astype(np.float16)
        in_maps.append({
            "hidT": hidT,
            "wqkv": w_local,
            "wo": wo_local,
            "cosb": cosB,
            "sinb": sinB,
            "r2t": R2T,
            "maskbig": maskbig,
        })
    return in_maps


def kernel(hidden_states, cos, sin, w_qkv, w_o):
    global _LAST_RESULTS
    nc = build_program(with_rs=True)
    in_maps = make_in_maps(hidden_states, cos, sin, w_qkv, w_o)
    res = run_bass_kernel_spmd(
        nc, in_maps, list(range(8)), trace=TRACE, **TRACE_KW
    )
    _LAST_RESULTS = res
    out = np.empty((2, S, HID), dtype=np.float32)
    for b in range(2):
        finT = np.empty((HID, S), dtype=np.float32)
        for g in range(4):
            yc = res.results[4 * b + g]["y"]  # (4, 512, 512)
            for i in range(NT):
                finT[TC * g:TC * (g + 1), TC * i:TC * (i + 1)] = yc[i]
        out[b] = finT.T
    return out
